# revision 8
# baseline (speedup 1.0000x reference)
"""Trainium2 Bass kernel for nn_Embed_38766374814290 (embedding_lookup).

Math: out[i,j,l,e] = A[m][e] + delta_s[i,j,l] * B[m][e]
  where m = (j < traj_len[i]), delta_s = where(m, mat2[traj_loc-1], 0),
  A[m] = emb_sl_w[m] + emb_tl_w[m],
  B[m] = (emb_su_w[m]-emb_sl_w[m])/SU + (emb_tu_w[m]-emb_tl_w[m])/TU.

Sharding: pure data parallel over batch N = 32 -> 4 rows per core x 8 cores.

The rel-err gate is 2e-2; bf16 output rounding is ~2^-9, so the device
computes and stores the output in bf16 (halving HBM write traffic vs
fp32 -> ~23us DMA roofline per core) and the host upcasts to fp32.

Per-core pipeline, per batch row i (128 positions):
  1. One transposing dma_gather pulls 128 rows of a padded 256-wide
     table (window g of a row = [mat2x[loc, 32g:32g+32] | m-mark | 1 |
     0-pad] at cols 64g:64g+64; invalid positions redirect to row 4096
     whose data and m-mark are 0 but 1-marks are 1). The xbar transpose
     lands it as gt[p, h, q] = table[idx[q]][128h + p]: partitions = l,
     free = positions -- lhsT layout directly, no on-chip transposes.
  2. Matmul per (g, s): lhsT = gt[64*(g&1) : +34, g>>1, :] (K=34 incl
     the m/1 rows -> A[m] added in-matmul), rhs = block-diagonal B1
     tiles (duplicated at partition base 64 for odd windows). Four
     s-matmuls (F=512) fill a [128, 2048] 4-bank PSUM tile per group.
  3. One wide [128, 2048] PSUM->SBUF eviction per (i, g) casts fp32 to
     bf16 (alternating Activation/Vector engines).
  4. Output DMA per (i, g): natural [pos, l*E] layout, 128 x 4KiB
     contiguous descriptors, spread over all 16 DMA queues.
"""
import os
import numpy as np
from contextlib import ExitStack

SU, TU = 10000.0, 86400.0
N, M, L, E = 32, 128, 128, 64
NLOC = 4096
NCORES = 8
ROWS = N // NCORES  # 4 batch rows per core

_CACHE = {}


def _install_profhook():
    """Optional: shim the missing antenv.axon_hooks so trace=True works."""
    import sys
    import types
    if "antenv.axon_hooks" in sys.modules:
        return True
    try:
        from trn_agent_boot.trn_boot import _ntff_profile_via_ctypes
    except Exception:
        return False
    hook = [None]
    mod = types.ModuleType("antenv.axon_hooks")
    mod.set_axon_ntff_profile_hook = lambda h: hook.__setitem__(0, h)
    mod.get_axon_ntff_profile_hook = lambda: hook[0]
    sys.modules["antenv.axon_hooks"] = mod
    try:
        mod.set_axon_ntff_profile_hook(
            _ntff_profile_via_ctypes("/opt/axon/libaxon_pjrt.so"))
    except Exception:
        return False
    return True


def _build():
    import concourse.bass as bass
    import concourse.tile as tile
    from concourse import bacc, mybir

    F32 = mybir.dt.float32
    BF16 = mybir.dt.bfloat16
    I16 = mybir.dt.int16

    nc = bacc.Bacc("TRN2", target_bir_lowering=False, debug=False,
                   enable_asserts=True, num_devices=NCORES)
    m2_d = nc.dram_tensor("m2", [NLOC + 1, 256], BF16,
                          kind="ExternalInput").ap()
    idx_d = nc.dram_tensor("idx", [128, 8 * ROWS], I16,
                           kind="ExternalInput").ap()
    rhs_d = nc.dram_tensor("rhs", [4, 34, 8 * E], BF16,
                           kind="ExternalInput").ap()
    out_d = nc.dram_tensor("out", [ROWS, M, L * E], BF16,
                           kind="ExternalOutput").ap()

    with tile.TileContext(nc) as tc, ExitStack() as ctx:
        const = ctx.enter_context(tc.tile_pool(name="const", bufs=1))
        gpool = ctx.enter_context(tc.tile_pool(name="gath", bufs=4))
        opool = ctx.enter_context(tc.tile_pool(name="orow", bufs=3))
        pso = ctx.enter_context(tc.tile_pool(name="pso", bufs=2, space="PSUM"))

        idxt = const.tile([128, 8 * ROWS], I16)
        nc.sync.dma_start(idxt[:], idx_d[:])
        # rhs tiles at partition base 0 (windows g=0,2) and 64 (g=1,3)
        rlo, rhi = [], []
        for s in range(4):
            rt = const.tile([34, 8 * E], BF16, tag=f"rlo{s}")
            nc.sync.dma_start(rt[:], rhs_d[s])
            rlo.append(rt)
        for s in range(4):
            rt = const.tile([98, 8 * E], BF16, tag=f"rhi{s}")
            nc.sync.dma_start(rt[64:98, :], rhs_d[s])
            rhi.append(rt)

        # HAM warmup: back-to-back matmuls lift the PE clock gate before
        # the real burst. Results are never read.
        wlhs = const.tile([128, 128], BF16)
        nc.vector.memset(wlhs[:], 0.0)
        wrhs = const.tile([128, 8 * E], BF16)
        nc.vector.memset(wrhs[:], 0.0)
        wpo = pso.tile([128, 4 * 8 * E], F32, tag="po")
        for _ in range(8):
            nc.tensor.matmul(wpo[:, 0:512], lhsT=wlhs[:], rhs=wrhs[:],
                             start=True, stop=True)

        # all four transposing gathers upfront: Pool engine streams them
        # while the PE warms up; gt[i] holds row i's lhsT windows
        gts = []
        for i in range(ROWS):
            gt = gpool.tile([128, 2, 128], BF16)
            nc.gpsimd.dma_gather(
                out_ap=gt[:], in_ap=m2_d[:],
                idxs_ap=idxt[:, 8 * i:8 * (i + 1)],
                num_idxs=128, num_idxs_reg=128, elem_size=256,
                transpose=True)
            gts.append(gt)

        # evict engine pattern: 9 scalar(ACT) / 7 vector(DVE)
        epat = [1, 0, 1, 0, 1, 0, 1, 1, 0, 1, 0, 1, 0, 1, 1, 0]

        for i in range(ROWS):
            for g in range(4):
                base = 64 * (g & 1)
                h = g >> 1
                po = pso.tile([128, 4 * 8 * E], F32, tag="po")
                for s in range(4):
                    if base == 0:
                        nc.tensor.matmul(po[:, 512 * s:512 * (s + 1)],
                                         lhsT=gts[i][0:34, h, :],
                                         rhs=rlo[s][:],
                                         start=True, stop=True)
                    else:
                        nc.tensor.matmul(po[:, 512 * s:512 * (s + 1)],
                                         lhsT=gts[i][64:98, h, :],
                                         rhs=rhi[s][64:98, :],
                                         start=True, stop=True)
                orow = opool.tile([128, 4 * 8 * E], BF16)
                if epat[4 * i + g]:
                    nc.scalar.copy(out=orow[:], in_=po[:])
                else:
                    nc.vector.tensor_copy(out=orow[:], in_=po[:])
                nc.sync.dma_start(out_d[i][:, 2048 * g:2048 * (g + 1)],
                                  orow[:])
    nc.compile()
    return nc


def kernel(traj_loc, mat2, vec, traj_len, l_max, emb_sl_w, emb_su_w,
           emb_tl_w, emb_tu_w):
    import ml_dtypes
    from concourse import bass_utils

    BF = ml_dtypes.bfloat16
    traj_loc = np.asarray(traj_loc).astype(np.int64)
    mat2 = np.ascontiguousarray(np.asarray(mat2, dtype=np.float32))
    traj_len = np.asarray(traj_len).astype(np.int64)
    esl = np.asarray(emb_sl_w, dtype=np.float32)
    esu = np.asarray(emb_su_w, dtype=np.float32)
    etl = np.asarray(emb_tl_w, dtype=np.float32)
    etu = np.asarray(emb_tu_w, dtype=np.float32)

    # host prep: constants
    A = esl + etl                                            # [2, E]
    B = (esu - esl) / np.float32(SU) + (etu - etl) / np.float32(TU)
    mask = (np.arange(M)[None, :] < traj_len[:, None])       # [N, M]
    idx_full = np.where(mask, traj_loc - 1, NLOC).astype(np.int32)

    b1 = B[1].astype(BF)
    dA = (A[1] - A[0]).astype(BF)
    a0 = A[0].astype(BF)

    # padded gather table: window g = [ds 32 | m-mark | 1-mark | 0 pad]
    tab = np.zeros((NLOC + 1, 256), np.float32)
    for g in range(4):
        tab[:NLOC, 64 * g:64 * g + 32] = mat2[:, 32 * g:32 * (g + 1)]
        tab[:NLOC, 64 * g + 32] = 1.0   # m-marker: 0 in the zero-row
        tab[:, 64 * g + 33] = 1.0       # 1-marker: 1 everywhere
    tabBF = np.ascontiguousarray(tab.astype(BF))

    # idx wrapped in 16 partitions: idx16[p, 8i+s] = idx[i][16s + p%16]
    idx16 = np.empty((NCORES, 128, 8 * ROWS), np.int16)
    p16 = np.arange(128) % 16
    for c in range(NCORES):
        for i in range(ROWS):
            idx = idx_full[ROWS * c + i]
            for s in range(8):
                idx16[c, :, 8 * i + s] = idx[16 * s + p16]

    # rhs[s] is [34, 512]: row 8s+lp has B1 in e-block lp; rows 32/33
    # pair with lhsT rows [m, 1]: out += m*dA + A0 in every e-block.
    rhs = np.zeros((4, 34, 8 * E), BF)
    for s in range(4):
        for lp in range(8):
            rhs[s, 8 * s + lp, E * lp:E * (lp + 1)] = b1
        rhs[s, 32, :] = np.tile(dA, 8)
        rhs[s, 33, :] = np.tile(a0, 8)

    if "nc" not in _CACHE:
        _CACHE["nc"] = _build()
    nc = _CACHE["nc"]

    in_maps = []
    for c in range(NCORES):
        in_maps.append({
            "m2": tabBF,
            "idx": np.ascontiguousarray(idx16[c]),
            "rhs": rhs,
        })

    trace = os.environ.get("KERNEL_TRACE", "0") == "1" and _install_profhook()
    res = bass_utils.run_bass_kernel_spmd(
        nc, in_maps, core_ids=list(range(NCORES)), trace=bool(trace))
    if trace:
        _CACHE["exec_time_ns"] = res.exec_time_ns
        _CACHE["trace_path"] = (res.instructions_and_trace or (None, None))[1]
        _CACHE["tmpdir"] = res.profile_json

    out = np.concatenate(
        [res.results[c]["out"].reshape(ROWS, M, L, E) for c in range(NCORES)],
        axis=0).astype(np.float32)
    return out


# revision 10
# speedup vs baseline: 1.0155x; 1.0155x over previous
"""Trainium2 Bass kernel for nn_Embed_38766374814290 (embedding_lookup).

Math: out[i,j,l,e] = A[m][e] + delta_s[i,j,l] * B[m][e]
  where m = (j < traj_len[i]), delta_s = where(m, mat2[traj_loc-1], 0),
  A[m] = emb_sl_w[m] + emb_tl_w[m],
  B[m] = (emb_su_w[m]-emb_sl_w[m])/SU + (emb_tu_w[m]-emb_tl_w[m])/TU.

Sharding: pure data parallel over batch N = 32 -> 4 rows per core x 8 cores.

The rel-err gate is 2e-2; bf16 output rounding is ~2^-9, so the device
computes and stores the output in bf16 (halving HBM write traffic vs
fp32 -> ~23us DMA roofline per core) and the host upcasts to fp32.

Per-core pipeline, per batch row i (128 positions):
  1. One transposing dma_gather pulls 128 rows of a padded 256-wide
     table (window g of a row = [mat2x[loc, 32g:32g+32] | m-mark | 1 |
     0-pad] at cols 64g:64g+64; invalid positions redirect to row 4096
     whose data and m-mark are 0 but 1-marks are 1). The xbar transpose
     lands it as gt[p, h, q] = table[idx[q]][128h + p]: partitions = l,
     free = positions -- lhsT layout directly, no on-chip transposes.
  2. Matmul per (g, s): lhsT = gt[64*(g&1) : +34, g>>1, :] (K=34 incl
     the m/1 rows -> A[m] added in-matmul), rhs = block-diagonal B1
     tiles (duplicated at partition base 64 for odd windows). Four
     s-matmuls (F=512) fill a [128, 2048] 4-bank PSUM tile per group.
  3. One wide [128, 2048] PSUM->SBUF eviction per (i, g) casts fp32 to
     bf16 (alternating Activation/Vector engines).
  4. Output DMA per (i, g): natural [pos, l*E] layout, 128 x 4KiB
     contiguous descriptors, spread over all 16 DMA queues.
"""
import os
import numpy as np
from contextlib import ExitStack

SU, TU = 10000.0, 86400.0
N, M, L, E = 32, 128, 128, 64
NLOC = 4096
NCORES = 8
ROWS = N // NCORES  # 4 batch rows per core

_CACHE = {}


def _install_profhook():
    """Optional: shim the missing antenv.axon_hooks so trace=True works."""
    import sys
    import types
    if "antenv.axon_hooks" in sys.modules:
        return True
    try:
        from trn_agent_boot.trn_boot import _ntff_profile_via_ctypes
    except Exception:
        return False
    hook = [None]
    mod = types.ModuleType("antenv.axon_hooks")
    mod.set_axon_ntff_profile_hook = lambda h: hook.__setitem__(0, h)
    mod.get_axon_ntff_profile_hook = lambda: hook[0]
    sys.modules["antenv.axon_hooks"] = mod
    try:
        mod.set_axon_ntff_profile_hook(
            _ntff_profile_via_ctypes("/opt/axon/libaxon_pjrt.so"))
    except Exception:
        return False
    return True


def _build():
    import concourse.bass as bass
    import concourse.tile as tile
    from concourse import bacc, mybir

    F32 = mybir.dt.float32
    BF16 = mybir.dt.bfloat16
    I16 = mybir.dt.int16

    nc = bacc.Bacc("TRN2", target_bir_lowering=False, debug=False,
                   enable_asserts=True, num_devices=NCORES)
    m2_d = nc.dram_tensor("m2", [NLOC + 1, 256], BF16,
                          kind="ExternalInput").ap()
    idx_d = nc.dram_tensor("idx", [128, 8 * ROWS], I16,
                           kind="ExternalInput").ap()
    rhs_d = nc.dram_tensor("rhs", [4, 34, 8 * E], BF16,
                           kind="ExternalInput").ap()
    out_d = nc.dram_tensor("out", [ROWS, M, L * E], BF16,
                           kind="ExternalOutput").ap()

    with tile.TileContext(nc) as tc, ExitStack() as ctx:
        const = ctx.enter_context(tc.tile_pool(name="const", bufs=1))
        gpool = ctx.enter_context(tc.tile_pool(name="gath", bufs=4))
        opool = ctx.enter_context(tc.tile_pool(name="orow", bufs=3))
        pso = ctx.enter_context(tc.tile_pool(name="pso", bufs=2, space="PSUM"))

        idxt = const.tile([128, 8 * ROWS], I16)
        nc.sync.dma_start(idxt[:], idx_d[:])

        # all four transposing gathers upfront: Pool engine streams them
        # while the PE warms up; gt[i] holds row i's lhsT windows
        gts = []
        for i in range(ROWS):
            gt = gpool.tile([128, 2, 128], BF16)
            nc.gpsimd.dma_gather(
                out_ap=gt[:], in_ap=m2_d[:],
                idxs_ap=idxt[:, 8 * i:8 * (i + 1)],
                num_idxs=128, num_idxs_reg=128, elem_size=256,
                transpose=True)
            gts.append(gt)

        # rhs tiles at partition base 0 (windows g=0,2) and 64 (g=1,3)
        rlo, rhi = [], []
        for s in range(4):
            rt = const.tile([34, 8 * E], BF16, tag=f"rlo{s}")
            nc.scalar.dma_start(rt[:], rhs_d[s])
            rlo.append(rt)
        for s in range(4):
            rt = const.tile([98, 8 * E], BF16, tag=f"rhi{s}")
            nc.scalar.dma_start(rt[64:98, :], rhs_d[s])
            rhi.append(rt)

        # HAM warmup: back-to-back matmuls lift the PE clock gate before
        # the real burst. Results are never read.
        wlhs = const.tile([128, 128], BF16)
        nc.vector.memset(wlhs[:], 0.0)
        wrhs = const.tile([128, 8 * E], BF16)
        nc.vector.memset(wrhs[:], 0.0)
        wpo = pso.tile([128, 4 * 8 * E], F32, tag="po")
        for _ in range(14):
            nc.tensor.matmul(wpo[:, 0:512], lhsT=wlhs[:], rhs=wrhs[:],
                             start=True, stop=True)

        # evict engine pattern: 9 scalar(ACT) / 7 vector(DVE)
        epat = [1, 0, 1, 0, 1, 0, 1, 1, 0, 1, 0, 1, 0, 1, 1, 0]

        for i in range(ROWS):
            for g in range(4):
                base = 64 * (g & 1)
                h = g >> 1
                po = pso.tile([128, 4 * 8 * E], F32, tag="po")
                for s in range(4):
                    if base == 0:
                        nc.tensor.matmul(po[:, 512 * s:512 * (s + 1)],
                                         lhsT=gts[i][0:34, h, :],
                                         rhs=rlo[s][:],
                                         start=True, stop=True)
                    else:
                        nc.tensor.matmul(po[:, 512 * s:512 * (s + 1)],
                                         lhsT=gts[i][64:98, h, :],
                                         rhs=rhi[s][64:98, :],
                                         start=True, stop=True)
                orow = opool.tile([128, 4 * 8 * E], BF16)
                if epat[4 * i + g]:
                    nc.scalar.copy(out=orow[:], in_=po[:])
                else:
                    nc.vector.tensor_copy(out=orow[:], in_=po[:])
                nc.sync.dma_start(out_d[i][:, 2048 * g:2048 * (g + 1)],
                                  orow[:])
    nc.compile()
    return nc


def kernel(traj_loc, mat2, vec, traj_len, l_max, emb_sl_w, emb_su_w,
           emb_tl_w, emb_tu_w):
    import ml_dtypes
    from concourse import bass_utils

    BF = ml_dtypes.bfloat16
    traj_loc = np.asarray(traj_loc).astype(np.int64)
    mat2 = np.ascontiguousarray(np.asarray(mat2, dtype=np.float32))
    traj_len = np.asarray(traj_len).astype(np.int64)
    esl = np.asarray(emb_sl_w, dtype=np.float32)
    esu = np.asarray(emb_su_w, dtype=np.float32)
    etl = np.asarray(emb_tl_w, dtype=np.float32)
    etu = np.asarray(emb_tu_w, dtype=np.float32)

    # host prep: constants
    A = esl + etl                                            # [2, E]
    B = (esu - esl) / np.float32(SU) + (etu - etl) / np.float32(TU)
    mask = (np.arange(M)[None, :] < traj_len[:, None])       # [N, M]
    idx_full = np.where(mask, traj_loc - 1, NLOC).astype(np.int32)

    b1 = B[1].astype(BF)
    dA = (A[1] - A[0]).astype(BF)
    a0 = A[0].astype(BF)

    # padded gather table: window g = [ds 32 | m-mark | 1-mark | 0 pad]
    tab = np.zeros((NLOC + 1, 256), np.float32)
    for g in range(4):
        tab[:NLOC, 64 * g:64 * g + 32] = mat2[:, 32 * g:32 * (g + 1)]
        tab[:NLOC, 64 * g + 32] = 1.0   # m-marker: 0 in the zero-row
        tab[:, 64 * g + 33] = 1.0       # 1-marker: 1 everywhere
    tabBF = np.ascontiguousarray(tab.astype(BF))

    # idx wrapped in 16 partitions: idx16[p, 8i+s] = idx[i][16s + p%16]
    idx16 = np.empty((NCORES, 128, 8 * ROWS), np.int16)
    p16 = np.arange(128) % 16
    for c in range(NCORES):
        for i in range(ROWS):
            idx = idx_full[ROWS * c + i]
            for s in range(8):
                idx16[c, :, 8 * i + s] = idx[16 * s + p16]

    # rhs[s] is [34, 512]: row 8s+lp has B1 in e-block lp; rows 32/33
    # pair with lhsT rows [m, 1]: out += m*dA + A0 in every e-block.
    rhs = np.zeros((4, 34, 8 * E), BF)
    for s in range(4):
        for lp in range(8):
            rhs[s, 8 * s + lp, E * lp:E * (lp + 1)] = b1
        rhs[s, 32, :] = np.tile(dA, 8)
        rhs[s, 33, :] = np.tile(a0, 8)

    if "nc" not in _CACHE:
        _CACHE["nc"] = _build()
    nc = _CACHE["nc"]

    in_maps = []
    for c in range(NCORES):
        in_maps.append({
            "m2": tabBF,
            "idx": np.ascontiguousarray(idx16[c]),
            "rhs": rhs,
        })

    trace = os.environ.get("KERNEL_TRACE", "0") == "1" and _install_profhook()
    res = bass_utils.run_bass_kernel_spmd(
        nc, in_maps, core_ids=list(range(NCORES)), trace=bool(trace))
    if trace:
        _CACHE["exec_time_ns"] = res.exec_time_ns
        _CACHE["trace_path"] = (res.instructions_and_trace or (None, None))[1]
        _CACHE["tmpdir"] = res.profile_json

    out = np.concatenate(
        [res.results[c]["out"].reshape(ROWS, M, L, E) for c in range(NCORES)],
        axis=0).astype(np.float32)
    return out
